# revision 1
# baseline (speedup 1.0000x reference)
"""Trainium2 Bass kernel for nn_ContrastLoss.

Reference computation (B=128, P=256 proposals/image, D=1024, K=4 scales):
    box_n = l2norm(box.reshape(B,P,D));  z_n = l2norm(crop)      # [K,B,D]
    cos   = einsum('bpd,kbd->kbp', box_n, z_n)
    mask  = ious >= 0.4  (per (b,p));  cnt_pos = mask.sum(p)
    sim_pos = -(cos*mask).sum(p)/cnt_pos ; sim_neg = -(cos*~mask).sum(p)/cnt_neg
    L[k] = softplus((sim_neg-sim_pos)/T).sum(b);  out = min_k L / B

Key algebraic restructure (per batch b):
    arg[k,b] = (sim_neg-sim_pos)/T = z_n[k,b] . S[b]
    S[b,d]   = sum_p w[b,p] * box[b,p,d]
    w[b,p]   = invnorm[b,p] * (mask*(1/cnt_pos+1/cnt_neg) - 1/cnt_neg)/T
so the only heavy pass over the 128 MiB box tensor is one streaming read that
feeds (a) a row-wise sum-of-squares (ScalarE, fused accumulate) and (b) a
PE matmul contraction over proposals with a [128,16] weight matrix.

Sharding: data-parallel over batch. Core c handles batches [16c,16c+16)
(= rows [4096c, 4096c+4096) of box / ious, crop[:, 16c:16c+16, :]).
Each core returns the softplus arguments for its 16 batches; the host applies
softplus, sums across cores, takes min over k and divides by B.
"""

import contextlib
import os
import sys

if "/opt/trn_rl_repo" not in sys.path:
    sys.path.insert(0, "/opt/trn_rl_repo")

import numpy as np

import concourse.bacc as bacc
import concourse.mybir as mybir
import concourse.tile as tile
from concourse.bass_utils import run_bass_kernel_spmd

# Problem constants (hardcoded per harness contract).
B, P, D, K = 128, 256, 1024, 4
N_CORES = 8
B_CORE = B // N_CORES            # 16 batches per core
ROWS = B_CORE * P                # 4096 rows per core
NT = ROWS // 128                 # 32 row-tiles of 128 rows
N_CHUNKS = 8                     # DMA chunks of the box slice
TILES_PER_CHUNK = NT // N_CHUNKS # 4 row-tiles per 2 MiB chunk
IOU_THRES = 0.4
TEMP = 0.2

USE_F32R = os.environ.get("KERNEL_F32R", "1") == "1"
# debug bisection: 1=DMAs only, 2=+mask/cnt/coef, 3=+squares/weights,
# 4=+S matmuls, 5=full
STAGE = int(os.environ.get("KERNEL_STAGE", "5"))
# sub-steps within stage 3: 1=square+accum, 2=+recip/sqrt, 3=+weight write,
# 4=+z-norms
S3 = int(os.environ.get("KERNEL_S3", "4"))

F32 = mybir.dt.float32
F32R = mybir.dt.float32r if USE_F32R else mybir.dt.float32
BF16 = mybir.dt.bfloat16
AF = mybir.ActivationFunctionType
ALU = mybir.AluOpType


def _emit(tc):
    nc = tc.nc
    box = nc.dram_tensor("box", [ROWS, D], F32, kind="ExternalInput").ap()
    iou_t = nc.dram_tensor("iou_t", [128, NT], F32, kind="ExternalInput").ap()
    crop = nc.dram_tensor("crop", [K, B_CORE, D], F32, kind="ExternalInput").ap()
    zeros_in = nc.dram_tensor(
        "zeros_in", [128, NT * B_CORE], F32, kind="ExternalInput"
    ).ap()
    out_l = nc.dram_tensor("out_l", [B_CORE, K], F32, kind="ExternalOutput").ap()

    ctx = contextlib.ExitStack()
    with ctx:
        const = ctx.enter_context(tc.tile_pool(name="const", bufs=1))
        boxpool = ctx.enter_context(tc.tile_pool(name="boxpool", bufs=N_CHUNKS))
        sqpool = ctx.enter_context(tc.tile_pool(name="sqpool", bufs=2))
        smpool = ctx.enter_context(tc.tile_pool(name="smpool", bufs=4))
        psS = ctx.enter_context(tc.tile_pool(name="psS", bufs=1, space="PSUM"))
        psmisc = ctx.enter_context(tc.tile_pool(name="psmisc", bufs=1, space="PSUM"))

        # --- small inputs -------------------------------------------------
        iou_sb = const.tile([128, NT], F32)
        nc.sync.dma_start(iou_sb[:], iou_t[:])
        z_sb = const.tile([16, K * D], F32)
        for k in range(K):
            nc.sync.dma_start(z_sb[:, k * D:(k + 1) * D], crop[k, :, :])

        # --- box chunk DMAs (streaming) -----------------------------------
        CH_COLS = TILES_PER_CHUNK * D
        box3 = box.rearrange("(t p) d -> p t d", p=128)
        chunks = []
        for c in range(N_CHUNKS):
            ch = boxpool.tile([128, CH_COLS], F32R, name=f"ch{c}", tag="ch")
            ch3 = ch.rearrange("p (t d) -> p t d", d=D)
            src = box3[:, c * TILES_PER_CHUNK:(c + 1) * TILES_PER_CHUNK, :]
            if USE_F32R:
                src = src.bitcast(F32R)
            nc.sync.dma_start(ch3, src)
            chunks.append(ch)

        coef_bc = None
        mask = None
        if STAGE >= 2:
            # bf16 for the tiny count/broadcast matmuls: walrus codegen
            # rejects the fp32 lowering of K=1/M=1 matmuls, and bf16 is
            # exact for ones/0-1 masks while coef rounding (~4e-3) is far
            # below tolerance.
            ones_col = const.tile([128, 1], BF16)
            nc.vector.memset(ones_col[:], 1.0)
            ones_row = const.tile([1, 128], BF16)
            nc.vector.memset(ones_row[:], 1.0)

            # mask[p, t] = iou >= thres  (1.0 / 0.0)
            mask = const.tile([128, NT], BF16)
            nc.vector.tensor_scalar(mask[:], iou_sb[:], IOU_THRES, None, ALU.is_ge)

            # cnt per row-tile column: ones[128,1].T @ mask -> [1, NT]
            ps_cnt = psmisc.tile([1, NT], F32)
            nc.tensor.matmul(ps_cnt[:], ones_col[:], mask[:], start=True, stop=True)

            cnt_t = const.tile([1, NT], F32)
            nc.vector.tensor_copy(cnt_t[:], ps_cnt[:])
            cnt_pos = const.tile([1, B_CORE], F32)
            nc.vector.tensor_tensor(
                cnt_pos[:], cnt_t[0:1, 0:NT:2], cnt_t[0:1, 1:NT:2], ALU.add
            )
            rcp_p = const.tile([1, B_CORE], F32)
            nc.vector.reciprocal(rcp_p[:], cnt_pos[:])
            cnt_neg = const.tile([1, B_CORE], F32)
            nc.vector.tensor_scalar(
                cnt_neg[:], cnt_pos[:], -1.0, float(P), ALU.mult, ALU.add
            )
            rcp_n = const.tile([1, B_CORE], F32)
            nc.vector.reciprocal(rcp_n[:], cnt_neg[:])

            # coefA=(rcp_p+rcp_n)/T at cols 2b,2b+1 ; coefB=rcp_n/T at NT+...
            coef_row = const.tile([1, 2 * NT], BF16)
            tmp_ab = const.tile([1, B_CORE], F32)
            nc.vector.tensor_tensor(tmp_ab[:], rcp_p[:], rcp_n[:], ALU.add)
            for rep in range(2):
                nc.vector.tensor_scalar(
                    coef_row[0:1, rep:NT:2], tmp_ab[:], 1.0 / TEMP, None, ALU.mult
                )
                nc.vector.tensor_scalar(
                    coef_row[0:1, NT + rep:2 * NT:2], rcp_n[:], 1.0 / TEMP,
                    None, ALU.mult,
                )

            # broadcast to all 128 partitions: ones[1,128].T @ coef[1,2NT]
            ps_coef = psmisc.tile([128, 2 * NT], F32)
            nc.tensor.matmul(
                ps_coef[:], ones_row[:], coef_row[:], start=True, stop=True
            )
            coef_bc = const.tile([128, 2 * NT], F32)
            nc.vector.tensor_copy(coef_bc[:], ps_coef[:])

        w_sp = None
        if STAGE >= 3:
            # sparse per-tile weight columns: w_sp[:, 16*t + t//2] nonzero
            # (float32r so the fp32r matmul sees pre-rounded producers;
            # zeroed via DMA because Memset cannot emit float32r)
            w_sp = const.tile([128, NT * B_CORE], F32R)
            zsrc = zeros_in[:]
            if USE_F32R:
                zsrc = zsrc.bitcast(F32R)
            nc.sync.dma_start(w_sp[:], zsrc)

        ps_S = psS.tile([B_CORE, D], F32)
        TPC = TILES_PER_CHUNK
        ss_all = const.tile([128, NT], F32)

        # --- main streaming pass over box ---------------------------------
        # Per chunk: 4 ACT squares (fused row sum-of-squares), then ONE
        # batched recip/sqrt/weight computation for the 4 columns so ACT
        # never stalls per-tile on the DVE round trip, then 8 matmuls.
        for c in range(N_CHUNKS):
            ch = chunks[c]
            t0 = c * TPC
            if STAGE >= 3:
                for rt in range(TPC):
                    t = t0 + rt
                    btile = ch[:, rt * D:(rt + 1) * D]
                    if USE_F32R:
                        btile = btile.bitcast(F32)
                    sq = sqpool.tile([128, D], F32, name="sq", tag="sq")
                    nc.scalar.activation(
                        sq[:], btile, AF.Square, accum_out=ss_all[:, t:t + 1]
                    )
                if S3 >= 2:
                    rec4 = smpool.tile([128, TPC], F32, name="rec4", tag="rec")
                    nc.vector.reciprocal(rec4[:], ss_all[:, t0:t0 + TPC])
                    invn4 = smpool.tile([128, TPC], F32, name="invn4", tag="invn")
                    nc.scalar.activation(invn4[:], rec4[:], AF.Sqrt)
                if S3 >= 3:
                    wt4 = smpool.tile([128, TPC], F32, name="wt4", tag="wtmp")
                    nc.vector.tensor_tensor(
                        wt4[:], mask[:, t0:t0 + TPC], coef_bc[:, t0:t0 + TPC],
                        ALU.mult,
                    )
                    nc.vector.tensor_tensor(
                        wt4[:], wt4[:], coef_bc[:, NT + t0:NT + t0 + TPC],
                        ALU.subtract,
                    )
                    nc.vector.tensor_tensor(wt4[:], wt4[:], invn4[:], ALU.mult)
                    # scatter the 4 columns to w_sp[:, 16t + t//2]; same-parity
                    # t are 33 columns apart, so two strided copies cover it
                    for par in range(2):
                        t = t0 + par
                        col = t * B_CORE + t // 2
                        nc.vector.tensor_copy(
                            w_sp[:, col:col + 34:33], wt4[:, par:par + 3:2]
                        )
            if STAGE >= 4:
                for rt in range(TPC):
                    t = t0 + rt
                    lhsT = w_sp[:, t * B_CORE:(t + 1) * B_CORE]
                    for h in range(2):
                        nc.tensor.matmul(
                            ps_S[:, h * 512:(h + 1) * 512],
                            lhsT,
                            ch[:, rt * D + h * 512:rt * D + (h + 1) * 512],
                            start=(t == 0),
                            stop=(t == NT - 1),
                            skip_group_check=True,
                        )

        # --- z normalization (independent of box stream) ------------------
        inv_zn = None
        if STAGE >= 3 and S3 >= 4:
            zss = const.tile([16, K], F32)
            for k in range(K):
                zsq = sqpool.tile([16, D], F32, name="zsq", tag="sq")
                nc.vector.tensor_tensor(
                    zsq[:], z_sb[:, k * D:(k + 1) * D], z_sb[:, k * D:(k + 1) * D],
                    ALU.mult,
                )
                nc.vector.reduce_sum(
                    zss[:, k:k + 1], zsq[:], axis=mybir.AxisListType.X
                )
            zrec = const.tile([16, K], F32)
            nc.vector.reciprocal(zrec[:], zss[:])
            inv_zn = const.tile([16, K], F32)
            nc.scalar.activation(inv_zn[:], zrec[:], AF.Sqrt)

        # --- final dots, scaled by z invnorm ------------------------------
        args = const.tile([16, K], F32)
        if STAGE >= 5:
            dots = const.tile([16, K], F32)
            for k in range(K):
                dsc = sqpool.tile([16, D], F32, name="dsc", tag="sq")
                nc.vector.tensor_tensor(
                    dsc[:], z_sb[:, k * D:(k + 1) * D], ps_S[:], ALU.mult
                )
                nc.vector.reduce_sum(
                    dots[:, k:k + 1], dsc[:], axis=mybir.AxisListType.X
                )
            nc.vector.tensor_tensor(args[:], dots[:], inv_zn[:], ALU.mult)
        elif STAGE == 4:
            nc.vector.tensor_copy(args[:], ps_S[:, 0:K])
        elif STAGE == 3:
            nc.vector.tensor_copy(args[:], w_sp[0:16, 0:K].bitcast(F32))
        elif STAGE == 2:
            nc.vector.tensor_copy(args[:], coef_bc[0:16, 0:K])
        else:
            nc.vector.tensor_copy(args[:], z_sb[:, 0:K])
        # softplus + batch-sum + min over k happen on the host (512 scalars)
        nc.sync.dma_start(out_l[:], args[:])


_NC_CACHE = None


def _get_nc():
    global _NC_CACHE
    if _NC_CACHE is None:
        nc = bacc.Bacc(
            "TRN2", target_bir_lowering=False, debug=False, num_devices=N_CORES
        )
        with tile.TileContext(nc) as tc:
            _emit(tc)
        nc.compile()
        _NC_CACHE = nc
    return _NC_CACHE


def _in_maps(box_cls_feat_con, crop_feat_con, ious):
    box = np.ascontiguousarray(np.asarray(box_cls_feat_con, dtype=np.float32))
    crop = np.ascontiguousarray(np.asarray(crop_feat_con, dtype=np.float32))
    iou = np.asarray(ious, dtype=np.float32)
    maps = []
    for c in range(N_CORES):
        rows = slice(c * ROWS, (c + 1) * ROWS)
        bsl = slice(c * B_CORE, (c + 1) * B_CORE)
        maps.append({
            "box": np.ascontiguousarray(box[rows]),
            "iou_t": np.ascontiguousarray(iou[rows].reshape(NT, 128).T),
            "crop": np.ascontiguousarray(crop[:, bsl, :]),
            "zeros_in": np.zeros((128, NT * B_CORE), dtype=np.float32),
        })
    return maps


def kernel(box_cls_feat_con, crop_feat_con, batch_size, ious, _trace=False):
    nc = _get_nc()
    maps = _in_maps(box_cls_feat_con, crop_feat_con, ious)
    res = run_bass_kernel_spmd(nc, maps, core_ids=list(range(N_CORES)), trace=_trace)
    l_total = np.zeros(K, dtype=np.float64)
    for c in range(N_CORES):
        args = res.results[c]["out_l"].astype(np.float64)  # [B_CORE, K]
        l_total += np.log1p(np.exp(args)).sum(axis=0)
    out = np.float32(l_total.min() / float(B))
    if _trace:
        kernel._last_results = res
    return np.asarray(out, dtype=np.float32)



# revision 4
# speedup vs baseline: 1.5679x; 1.5679x over previous
"""Trainium2 Bass kernel for nn_ContrastLoss.

Reference computation (B=128, P=256 proposals/image, D=1024, K=4 scales):
    box_n = l2norm(box.reshape(B,P,D));  z_n = l2norm(crop)      # [K,B,D]
    cos   = einsum('bpd,kbd->kbp', box_n, z_n)
    mask  = ious >= 0.4  (per (b,p));  cnt_pos = mask.sum(p)
    sim_pos = -(cos*mask).sum(p)/cnt_pos ; sim_neg = -(cos*~mask).sum(p)/cnt_neg
    L[k] = softplus((sim_neg-sim_pos)/T).sum(b);  out = min_k L / B

Key algebraic restructure (per batch b):
    arg[k,b] = (sim_neg-sim_pos)/T = z_n[k,b] . S[b]
    S[b,d]   = sum_p w[b,p] * box[b,p,d]
    w[b,p]   = invnorm[b,p] * (mask*(1/cnt_pos+1/cnt_neg) - 1/cnt_neg)/T
so the only heavy pass over the 128 MiB box tensor is one streaming read that
feeds (a) a row-wise sum-of-squares (ScalarE, fused accumulate) and (b) a
PE matmul contraction over proposals with a [128,16] weight matrix.

The device computes S only; the z-normalization, z.S dots, softplus, batch
sum and min over k are a few hundred KFLOP finished on the host (like the
softplus/min in the original version). That keeps the device critical path
a single streamed pass over box: per 128-row tile (tile t = batch t//2),
DMA -> ACT square+accumulate -> (per batch) DVE recip / ACT sqrt / DVE
weight build+scatter -> 2 PE matmuls, everything overlapping the DMA
stream; the only post-stream work is the last batch's tail plus one
PSUM->SBUF eviction and a 64 KiB output DMA.

Sharding: data-parallel over batch. Core c handles batches [16c,16c+16)
(= rows [4096c, 4096c+4096) of box / ious) and returns S for its 16 batches.
"""

import contextlib
import sys

if "/opt/trn_rl_repo" not in sys.path:
    sys.path.insert(0, "/opt/trn_rl_repo")

import numpy as np

import concourse.bacc as bacc
import concourse.mybir as mybir
import concourse.tile as tile
from concourse.bass_utils import run_bass_kernel_spmd

# Problem constants (hardcoded per harness contract).
B, P, D, K = 128, 256, 1024, 4
N_CORES = 8
B_CORE = B // N_CORES            # 16 batches per core
ROWS = B_CORE * P                # 4096 rows per core
NT = ROWS // 128                 # 32 row-tiles of 128 rows; tile t = batch t//2
IOU_THRES = 0.4
TEMP = 0.2
EPS = 1e-12

F32 = mybir.dt.float32
F32R = mybir.dt.float32r
BF16 = mybir.dt.bfloat16
AF = mybir.ActivationFunctionType
ALU = mybir.AluOpType


def _emit(tc):
    nc = tc.nc
    box = nc.dram_tensor("box", [ROWS, D], F32, kind="ExternalInput").ap()
    iou_t = nc.dram_tensor("iou_t", [128, NT], F32, kind="ExternalInput").ap()
    out_S = nc.dram_tensor("out_S", [B_CORE, D], F32, kind="ExternalOutput").ap()

    ctx = contextlib.ExitStack()
    with ctx:
        const = ctx.enter_context(tc.tile_pool(name="const", bufs=1))
        boxpool = ctx.enter_context(tc.tile_pool(name="boxpool", bufs=12))
        sqpool = ctx.enter_context(tc.tile_pool(name="sqpool", bufs=2))
        smpool = ctx.enter_context(tc.tile_pool(name="smpool", bufs=6))
        psS = ctx.enter_context(tc.tile_pool(name="psS", bufs=1, space="PSUM"))
        psmisc = ctx.enter_context(tc.tile_pool(name="psmisc", bufs=1, space="PSUM"))

        # --- DMAs first so the box stream owns the queue from t=0 ---------
        iou_sb = const.tile([128, NT], F32)
        nc.sync.dma_start(iou_sb[:], iou_t[:])
        tiles = []
        for t in range(NT):
            bt = boxpool.tile([128, D], F32R, name=f"bt{t}", tag="bt")
            nc.sync.dma_start(bt[:], box[t * 128:(t + 1) * 128, :].bitcast(F32R))
            tiles.append(bt)

        # sparse per-tile weight columns: w_sp[:, 16*t + t//2] nonzero.
        # float32r so the fp32r matmul sees pre-rounded producers (the BIR
        # verifier rejects f32 writers into f32r matmul operands); Memset
        # cannot emit float32r, so zero via a DVE copy-convert from a
        # memset f32 scratch — off the DMA queue, unlike the old DMA fill.
        w_sp = const.tile([128, NT * B_CORE], F32R)
        w_zero = const.tile([128, NT * B_CORE], F32)
        nc.vector.memset(w_zero[:], 0.0)
        nc.vector.tensor_copy(w_sp[:], w_zero[:])

        # --- mask / counts / coefficients ---------------------------------
        # bf16 for the tiny count/broadcast matmuls: walrus codegen rejects
        # the fp32 lowering of K=1/M=1 matmuls, and bf16 is exact for
        # ones/0-1 masks while coef rounding (~4e-3) is far below tolerance.
        ones_col = const.tile([128, 1], BF16)
        nc.vector.memset(ones_col[:], 1.0)
        ones_row = const.tile([1, 128], BF16)
        nc.vector.memset(ones_row[:], 1.0)

        # mask[p, t] = iou >= thres  (1.0 / 0.0)
        mask = const.tile([128, NT], BF16)
        nc.vector.tensor_scalar(mask[:], iou_sb[:], IOU_THRES, None, ALU.is_ge)

        # cnt per row-tile column: ones[128,1].T @ mask -> [1, NT]
        ps_cnt = psmisc.tile([1, NT], F32)
        nc.tensor.matmul(ps_cnt[:], ones_col[:], mask[:], start=True, stop=True)

        cnt_t = const.tile([1, NT], F32)
        nc.vector.tensor_copy(cnt_t[:], ps_cnt[:])
        cnt_pos = const.tile([1, B_CORE], F32)
        nc.vector.tensor_tensor(
            cnt_pos[:], cnt_t[0:1, 0:NT:2], cnt_t[0:1, 1:NT:2], ALU.add
        )
        rcp_p = const.tile([1, B_CORE], F32)
        nc.vector.reciprocal(rcp_p[:], cnt_pos[:])
        cnt_neg = const.tile([1, B_CORE], F32)
        nc.vector.tensor_scalar(
            cnt_neg[:], cnt_pos[:], -1.0, float(P), ALU.mult, ALU.add
        )
        rcp_n = const.tile([1, B_CORE], F32)
        nc.vector.reciprocal(rcp_n[:], cnt_neg[:])

        # coefA=(rcp_p+rcp_n)/T at cols 2b,2b+1 ; coefB=rcp_n/T at NT+...
        coef_row = const.tile([1, 2 * NT], BF16)
        tmp_ab = const.tile([1, B_CORE], F32)
        nc.vector.tensor_tensor(tmp_ab[:], rcp_p[:], rcp_n[:], ALU.add)
        for rep in range(2):
            nc.vector.tensor_scalar(
                coef_row[0:1, rep:NT:2], tmp_ab[:], 1.0 / TEMP, None, ALU.mult
            )
            nc.vector.tensor_scalar(
                coef_row[0:1, NT + rep:2 * NT:2], rcp_n[:], 1.0 / TEMP,
                None, ALU.mult,
            )

        # broadcast to all 128 partitions: ones[1,128].T @ coef[1,2NT]
        ps_coef = psmisc.tile([128, 2 * NT], F32)
        nc.tensor.matmul(
            ps_coef[:], ones_row[:], coef_row[:], start=True, stop=True
        )
        coef_bc = const.tile([128, 2 * NT], F32)
        nc.vector.tensor_copy(coef_bc[:], ps_coef[:])

        # --- main streaming pass over box, one batch (= 2 tiles) at a time
        ss_all = const.tile([128, NT], F32)
        ps_S = psS.tile([B_CORE, D], F32)
        for bi in range(B_CORE):
            t0 = 2 * bi
            for j in range(2):
                bt = tiles[t0 + j]
                sq = sqpool.tile([128, D], F32, name="sq", tag="sq")
                nc.scalar.activation(
                    sq[:], bt[:].bitcast(F32), AF.Square,
                    accum_out=ss_all[:, t0 + j:t0 + j + 1],
                )
            rec2 = smpool.tile([128, 2], F32, name="rec2", tag="rec")
            nc.vector.reciprocal(rec2[:], ss_all[:, t0:t0 + 2])
            invn2 = smpool.tile([128, 2], F32, name="invn2", tag="invn")
            nc.scalar.activation(invn2[:], rec2[:], AF.Sqrt)
            wt2 = smpool.tile([128, 2], F32, name="wt2", tag="wt")
            nc.vector.tensor_tensor(
                wt2[:], mask[:, t0:t0 + 2], coef_bc[:, t0:t0 + 2], ALU.mult
            )
            nc.vector.tensor_tensor(
                wt2[:], wt2[:], coef_bc[:, NT + t0:NT + t0 + 2], ALU.subtract
            )
            nc.vector.tensor_tensor(wt2[:], wt2[:], invn2[:], ALU.mult)
            # scatter to w_sp[:, 16*t + bi] for t = 2bi, 2bi+1 (16 apart)
            col = t0 * B_CORE + bi
            nc.vector.tensor_copy(
                w_sp[:, col:col + B_CORE + 1:B_CORE], wt2[:]
            )
            for j in range(2):
                t = t0 + j
                lhsT = w_sp[:, t * B_CORE:(t + 1) * B_CORE]
                for h in range(2):
                    nc.tensor.matmul(
                        ps_S[:, h * 512:(h + 1) * 512],
                        lhsT,
                        tiles[t][:, h * 512:(h + 1) * 512],
                        start=(t == 0),
                        stop=(t == NT - 1),
                        skip_group_check=True,
                    )

        # --- evict S and ship it out (halves on ACT + DVE in parallel) ----
        S_sb = const.tile([B_CORE, D], F32)
        nc.scalar.activation(S_sb[:, 0:512], ps_S[:, 0:512], AF.Copy)
        nc.vector.tensor_copy(S_sb[:, 512:1024], ps_S[:, 512:1024])
        nc.sync.dma_start(out_S[:], S_sb[:])


_NC_CACHE = None


def _get_nc():
    global _NC_CACHE
    if _NC_CACHE is None:
        nc = bacc.Bacc(
            "TRN2", target_bir_lowering=False, debug=False, num_devices=N_CORES
        )
        with tile.TileContext(nc) as tc:
            _emit(tc)
        nc.compile()
        _NC_CACHE = nc
    return _NC_CACHE


def kernel(box_cls_feat_con, crop_feat_con, batch_size, ious, _trace=False):
    nc = _get_nc()
    box = np.ascontiguousarray(np.asarray(box_cls_feat_con, dtype=np.float32))
    iou = np.asarray(ious, dtype=np.float32)
    maps = []
    for c in range(N_CORES):
        rows = slice(c * ROWS, (c + 1) * ROWS)
        maps.append({
            "box": box[rows],
            "iou_t": np.ascontiguousarray(iou[rows].reshape(NT, 128).T),
        })
    res = run_bass_kernel_spmd(nc, maps, core_ids=list(range(N_CORES)), trace=_trace)
    S = np.concatenate(
        [np.asarray(res.results[c]["out_S"]) for c in range(N_CORES)], axis=0
    ).astype(np.float64)  # [B, D]
    z = np.asarray(crop_feat_con, dtype=np.float64)  # [K, B, D]
    z_n = z / np.clip(np.linalg.norm(z, axis=-1, keepdims=True), EPS, None)
    args = np.einsum("kbd,bd->kb", z_n, S)
    L = np.logaddexp(0.0, args).sum(axis=-1)  # softplus + sum over batches
    out = np.float32(L.min() / float(B))
    if _trace:
        kernel._last_results = res
    return np.asarray(out, dtype=np.float32)


# revision 34
# speedup vs baseline: 1.6241x; 1.0358x over previous
"""Trainium2 Bass kernel for nn_ContrastLoss.

Reference computation (B=128, P=256 proposals/image, D=1024, K=4 scales):
    box_n = l2norm(box.reshape(B,P,D));  z_n = l2norm(crop)      # [K,B,D]
    cos   = einsum('bpd,kbd->kbp', box_n, z_n)
    mask  = ious >= 0.4  (per (b,p));  cnt_pos = mask.sum(p)
    sim_pos = -(cos*mask).sum(p)/cnt_pos ; sim_neg = -(cos*~mask).sum(p)/cnt_neg
    L[k] = softplus((sim_neg-sim_pos)/T).sum(b);  out = min_k L / B

Key algebraic restructure (per batch b):
    arg[k,b] = (sim_neg-sim_pos)/T = z_n[k,b] . S[b]
    S[b,d]   = sum_p w[b,p] * box[b,p,d]
    w[b,p]   = invnorm[b,p] * (mask*(1/cnt_pos+1/cnt_neg) - 1/cnt_neg)/T
so the only heavy pass over the 128 MiB box tensor is one streaming read that
feeds (a) a row-wise sum-of-squares (ScalarE, fused accumulate) and (b) a
PE matmul contraction over proposals with a [128,16] weight matrix.

The device computes S only; the z-normalization, z.S dots, softplus, batch
sum and min over k are a few hundred KFLOP finished on the host (like the
softplus/min in the original version). That keeps the device critical path
a single streamed pass over box: per 128-row tile (tile t = batch t//2),
DMA -> ACT square+accumulate -> (per batch) DVE recip / ACT sqrt / DVE
weight build+scatter -> 2 PE matmuls, everything overlapping the DMA
stream; the only post-stream work is the last batch's tail plus one
PSUM->SBUF eviction and a 64 KiB output DMA.

Sharding: data-parallel over batch. Core c handles batches [16c,16c+16)
(= rows [4096c, 4096c+4096) of box / ious) and returns S for its 16 batches.
"""

import contextlib
import os
import sys

if "/opt/trn_rl_repo" not in sys.path:
    sys.path.insert(0, "/opt/trn_rl_repo")

import numpy as np

import concourse.bacc as bacc
import concourse.mybir as mybir
import concourse.tile as tile
from concourse.bass_utils import run_bass_kernel_spmd

# Problem constants (hardcoded per harness contract).
B, P, D, K = 128, 256, 1024, 4
N_CORES = 8
B_CORE = B // N_CORES            # 16 batches per core
ROWS = B_CORE * P                # 4096 rows per core
NT = ROWS // 128                 # 32 row-tiles of 128 rows; tile t = batch t//2
IOU_THRES = 0.4
TEMP = 0.2
EPS = 1e-12

USE_TTR = os.environ.get("K_TTR", "1") == "1"
USE_BIAS_SQRT = os.environ.get("K_BIAS_SQRT", "1") == "1"

F32 = mybir.dt.float32
F32R = mybir.dt.float32r
BF16 = mybir.dt.bfloat16
F16 = mybir.dt.float16
AF = mybir.ActivationFunctionType
ALU = mybir.AluOpType


def _emit(tc):
    nc = tc.nc
    box = nc.dram_tensor("box", [ROWS, D], F32, kind="ExternalInput").ap()
    iou_t = nc.dram_tensor("iou_t", [128, NT], F32, kind="ExternalInput").ap()
    # fp16 S output: halves the eviction DMA. S elements are O(0.05), so
    # fp16's ~5e-4 relative rounding is far inside the 2e-2 budget (host
    # does the final dots in float64). (walrus rejects matmul PSUM writes
    # at nonzero start partitions, so S stays [16, 1024] in one band.)
    out_S = nc.dram_tensor("out_S", [B_CORE, D], F32, kind="ExternalOutput").ap()

    ctx = contextlib.ExitStack()
    with ctx:
        const = ctx.enter_context(tc.tile_pool(name="const", bufs=1))
        boxpool = ctx.enter_context(tc.tile_pool(name="boxpool", bufs=12))
        sqpool = ctx.enter_context(tc.tile_pool(name="sqpool", bufs=2))
        smpool = ctx.enter_context(tc.tile_pool(name="smpool", bufs=6))
        psS = ctx.enter_context(tc.tile_pool(name="psS", bufs=1, space="PSUM"))
        psmisc = ctx.enter_context(tc.tile_pool(name="psmisc", bufs=1, space="PSUM"))

        # --- DMAs first so the box stream owns the queue from t=0 ---------
        # Box tiles lead (the stream is the critical resource); iou slots in
        # after tile 1 — the mask/coef preamble isn't needed until the first
        # batch's weights anyway. The last two tiles arrive as 4 quarter
        # DMAs each so their sum-of-squares can start before the full tile
        # has landed, shortening the post-stream tail.
        iou_sb = const.tile([128, NT], F32)
        tiles = []
        HW = D // 2
        # per-tile square split between ACT (leading columns) and DVE
        # (trailing columns); asymmetric for the last tile so DVE's faster
        # reduce covers only the very last-arriving bytes
        CUT = [HW] * NT
        CUT[NT - 1] = 640
        for t in range(NT):
            bt = boxpool.tile([128, D], F32R, name=f"bt{t}", tag="bt")
            src = box[t * 128:(t + 1) * 128, :].bitcast(F32R)
            if t >= NT - 2:
                # split the trailing tiles' DMAs at the square cut so each
                # engine's piece can start as soon as its columns land
                c = CUT[t]
                nc.sync.dma_start(bt[:, 0:c], src[:, 0:c])
                nc.sync.dma_start(bt[:, c:D], src[:, c:D])
            else:
                nc.sync.dma_start(bt[:], src)
            tiles.append(bt)
            if t == 1:
                nc.sync.dma_start(iou_sb[:], iou_t[:])

        # sparse per-tile weight columns: w_sp[:, 16*t + t//2] nonzero.
        # float32r so the fp32r matmul sees pre-rounded producers (the BIR
        # verifier rejects f32 writers into f32r matmul operands); Memset
        # cannot emit float32r, so zero via a DVE copy-convert from a
        # memset f32 scratch — off the DMA queue, unlike the old DMA fill.
        w_sp = const.tile([128, NT * B_CORE], F32R)
        w_zero = const.tile([128, NT * B_CORE], F32)
        nc.vector.memset(w_zero[:], 0.0)
        nc.vector.tensor_copy(w_sp[:], w_zero[:])

        # --- mask / counts / coefficients ---------------------------------
        # bf16 for the tiny count/broadcast matmuls: walrus codegen rejects
        # the fp32 lowering of K=1/M=1 matmuls, and bf16 is exact for
        # ones/0-1 masks while coef rounding (~4e-3) is far below tolerance.
        ones_col = const.tile([128, 1], BF16)
        nc.vector.memset(ones_col[:], 1.0)
        ones_row = const.tile([1, 128], BF16)
        nc.vector.memset(ones_row[:], 1.0)

        # mask[p, t] = iou >= thres  (1.0 / 0.0)
        mask = const.tile([128, NT], BF16)
        nc.vector.tensor_scalar(mask[:], iou_sb[:], IOU_THRES, None, ALU.is_ge)

        # cnt per row-tile column: ones[128,1].T @ mask -> [1, NT]
        ps_cnt = psmisc.tile([1, NT], F32)
        nc.tensor.matmul(ps_cnt[:], ones_col[:], mask[:], start=True, stop=True)

        cnt_t = const.tile([1, NT], F32)
        nc.vector.tensor_copy(cnt_t[:], ps_cnt[:])
        cnt_pos = const.tile([1, B_CORE], F32)
        nc.vector.tensor_tensor(
            cnt_pos[:], cnt_t[0:1, 0:NT:2], cnt_t[0:1, 1:NT:2], ALU.add
        )
        rcp_p = const.tile([1, B_CORE], F32)
        nc.vector.reciprocal(rcp_p[:], cnt_pos[:])
        cnt_neg = const.tile([1, B_CORE], F32)
        nc.vector.tensor_scalar(
            cnt_neg[:], cnt_pos[:], -1.0, float(P), ALU.mult, ALU.add
        )
        rcp_n = const.tile([1, B_CORE], F32)
        nc.vector.reciprocal(rcp_n[:], cnt_neg[:])

        # coefA=(rcp_p+rcp_n)/T at cols 2b,2b+1 ; coefB=rcp_n/T at NT+...
        coef_row = const.tile([1, 2 * NT], BF16)
        tmp_ab = const.tile([1, B_CORE], F32)
        nc.vector.tensor_tensor(tmp_ab[:], rcp_p[:], rcp_n[:], ALU.add)
        for rep in range(2):
            nc.vector.tensor_scalar(
                coef_row[0:1, rep:NT:2], tmp_ab[:], 1.0 / TEMP, None, ALU.mult
            )
            nc.vector.tensor_scalar(
                coef_row[0:1, NT + rep:2 * NT:2], rcp_n[:], 1.0 / TEMP,
                None, ALU.mult,
            )

        # broadcast to all 128 partitions: ones[1,128].T @ coef[1,2NT]
        ps_coef = psmisc.tile([128, 2 * NT], F32)
        nc.tensor.matmul(
            ps_coef[:], ones_row[:], coef_row[:], start=True, stop=True
        )
        coef_bc = const.tile([128, 2 * NT], F32)
        nc.vector.tensor_copy(coef_bc[:], ps_coef[:])

        # pre_w[p,t] = mask*coefA - coefB: everything but the invnorm factor,
        # computed once so the per-batch tail chain is a single multiply
        pre_w = const.tile([128, NT], F32)
        nc.vector.tensor_tensor(
            pre_w[:], mask[:], coef_bc[:, 0:NT], ALU.mult
        )
        nc.vector.tensor_tensor(
            pre_w[:], pre_w[:], coef_bc[:, NT:2 * NT], ALU.subtract
        )

        # --- main streaming pass over box, one tile at a time -------------
        # Per tile: sum-of-squares split across ACT (first half-D, fused
        # accum) and DVE (second half, fused tensor_tensor_reduce); the two
        # partials combine inside the ACT sqrt via its bias operand, so the
        # whole norm chain is sq(ACT)/sq(DVE) -> sqrt(ACT) -> recip(DVE) ->
        # weight TT(DVE) -> 2 matmuls, with single cross-engine hops.
        ssa = const.tile([128, NT], F32)
        ssb = const.tile([128, NT], F32)
        norm_all = const.tile([128, NT], F32)
        ps_S = psS.tile([B_CORE, D], F32)
        S_sb = const.tile([B_CORE, D], F32)
        for t in range(NT):
            bt = tiles[t]
            c = CUT[t]
            sq = sqpool.tile([128, 640], F32, name="sq", tag="sq")
            nc.scalar.activation(
                sq[:, 0:c], bt[:, 0:c].bitcast(F32), AF.Square,
                accum_out=ssa[:, t:t + 1],
            )
            sq2 = sqpool.tile([128, HW], F32, name="sq2", tag="sq2")
            if USE_TTR:
                nc.vector.tensor_tensor_reduce(
                    sq2[:, 0:D - c], bt[:, c:D].bitcast(F32), bt[:, c:D].bitcast(F32),
                    1.0, 0.0, ALU.mult, ALU.add, accum_out=ssb[:, t:t + 1],
                )
            else:
                nc.vector.tensor_tensor(
                    sq2[:, 0:D - c], bt[:, c:D].bitcast(F32), bt[:, c:D].bitcast(F32),
                    ALU.mult,
                )
                nc.vector.reduce_sum(
                    ssb[:, t:t + 1], sq2[:, 0:D - c], axis=mybir.AxisListType.X
                )
            # norm = sqrt(ssa + ssb): ssb rides in via the bias operand
            if USE_BIAS_SQRT:
                nc.scalar.activation(
                    norm_all[:, t:t + 1], ssa[:, t:t + 1], AF.Sqrt,
                    bias=ssb[:, t:t + 1],
                )
            else:
                nc.vector.tensor_tensor(
                    ssa[:, t:t + 1], ssa[:, t:t + 1], ssb[:, t:t + 1], ALU.add
                )
                nc.scalar.activation(
                    norm_all[:, t:t + 1], ssa[:, t:t + 1], AF.Sqrt
                )
            invn1 = smpool.tile([128, 1], F32, name="invn1", tag="invn")
            nc.vector.reciprocal(invn1[:], norm_all[:, t:t + 1])
            col = t * B_CORE + t // 2
            nc.vector.tensor_tensor(
                w_sp[:, col:col + 1], pre_w[:, t:t + 1], invn1[:], ALU.mult
            )
            lhsT = w_sp[:, t * B_CORE:(t + 1) * B_CORE]
            for h in range(2):
                nc.tensor.matmul(
                    ps_S[:, h * HW:(h + 1) * HW],
                    lhsT,
                    bt[:, h * HW:(h + 1) * HW],
                    start=(t == 0),
                    stop=(t == NT - 1),
                    skip_group_check=True,
                )

        # evict + one fp16 DMA
        nc.vector.tensor_copy(S_sb[:], ps_S[:])
        nc.sync.dma_start(out_S[:], S_sb[:])


_NC_CACHE = None


def _get_nc():
    global _NC_CACHE
    if _NC_CACHE is None:
        nc = bacc.Bacc(
            "TRN2", target_bir_lowering=False, debug=False, num_devices=N_CORES
        )
        with tile.TileContext(nc) as tc:
            _emit(tc)
        nc.compile()
        _NC_CACHE = nc
    return _NC_CACHE


def kernel(box_cls_feat_con, crop_feat_con, batch_size, ious, _trace=False):
    nc = _get_nc()
    box = np.ascontiguousarray(np.asarray(box_cls_feat_con, dtype=np.float32))
    iou = np.asarray(ious, dtype=np.float32)
    maps = []
    for c in range(N_CORES):
        rows = slice(c * ROWS, (c + 1) * ROWS)
        maps.append({
            "box": box[rows],
            "iou_t": np.ascontiguousarray(iou[rows].reshape(NT, 128).T),
        })
    res = run_bass_kernel_spmd(nc, maps, core_ids=list(range(N_CORES)), trace=_trace)
    S = np.concatenate(
        [np.asarray(res.results[c]["out_S"]) for c in range(N_CORES)], axis=0
    ).astype(np.float64)  # [B, D]
    z = np.asarray(crop_feat_con, dtype=np.float64)  # [K, B, D]
    z_n = z / np.clip(np.linalg.norm(z, axis=-1, keepdims=True), EPS, None)
    args = np.einsum("kbd,bd->kb", z_n, S)
    L = np.logaddexp(0.0, args).sum(axis=-1)  # softplus + sum over batches
    out = np.float32(L.min() / float(B))
    if _trace:
        kernel._last_results = res
    return np.asarray(out, dtype=np.float32)


# revision 35
# speedup vs baseline: 1.6326x; 1.0052x over previous
"""Trainium2 Bass kernel for nn_ContrastLoss.

Reference computation (B=128, P=256 proposals/image, D=1024, K=4 scales):
    box_n = l2norm(box.reshape(B,P,D));  z_n = l2norm(crop)      # [K,B,D]
    cos   = einsum('bpd,kbd->kbp', box_n, z_n)
    mask  = ious >= 0.4  (per (b,p));  cnt_pos = mask.sum(p)
    sim_pos = -(cos*mask).sum(p)/cnt_pos ; sim_neg = -(cos*~mask).sum(p)/cnt_neg
    L[k] = softplus((sim_neg-sim_pos)/T).sum(b);  out = min_k L / B

Key algebraic restructure (per batch b):
    arg[k,b] = (sim_neg-sim_pos)/T = z_n[k,b] . S[b]
    S[b,d]   = sum_p w[b,p] * box[b,p,d]
    w[b,p]   = invnorm[b,p] * (mask*(1/cnt_pos+1/cnt_neg) - 1/cnt_neg)/T
so the only heavy pass over the 128 MiB box tensor is one streaming read that
feeds (a) a row-wise sum-of-squares (ScalarE, fused accumulate) and (b) a
PE matmul contraction over proposals with a [128,16] weight matrix.

The device computes S only; the z-normalization, z.S dots, softplus, batch
sum and min over k are a few hundred KFLOP finished on the host (like the
softplus/min in the original version). That keeps the device critical path
a single streamed pass over box: per 128-row tile (tile t = batch t//2),
DMA -> ACT square+accumulate -> (per batch) DVE recip / ACT sqrt / DVE
weight build+scatter -> 2 PE matmuls, everything overlapping the DMA
stream; the only post-stream work is the last batch's tail plus one
PSUM->SBUF eviction and a 64 KiB output DMA.

Sharding: data-parallel over batch. Core c handles batches [16c,16c+16)
(= rows [4096c, 4096c+4096) of box / ious) and returns S for its 16 batches.
"""

import contextlib
import os
import sys

if "/opt/trn_rl_repo" not in sys.path:
    sys.path.insert(0, "/opt/trn_rl_repo")

import numpy as np

import concourse.bacc as bacc
import concourse.mybir as mybir
import concourse.tile as tile
from concourse.bass_utils import run_bass_kernel_spmd

# Problem constants (hardcoded per harness contract).
B, P, D, K = 128, 256, 1024, 4
N_CORES = 8
B_CORE = B // N_CORES            # 16 batches per core
ROWS = B_CORE * P                # 4096 rows per core
NT = ROWS // 128                 # 32 row-tiles of 128 rows; tile t = batch t//2
IOU_THRES = 0.4
TEMP = 0.2
EPS = 1e-12

USE_TTR = os.environ.get("K_TTR", "1") == "1"
USE_BIAS_SQRT = os.environ.get("K_BIAS_SQRT", "1") == "1"

F32 = mybir.dt.float32
F32R = mybir.dt.float32r
BF16 = mybir.dt.bfloat16
F16 = mybir.dt.float16
AF = mybir.ActivationFunctionType
ALU = mybir.AluOpType


def _emit(tc):
    nc = tc.nc
    box = nc.dram_tensor("box", [ROWS, D], F32, kind="ExternalInput").ap()
    iou_t = nc.dram_tensor("iou_t", [128, NT], F32, kind="ExternalInput").ap()
    # fp16 S output: halves the eviction DMA. S elements are O(0.05), so
    # fp16's ~5e-4 relative rounding is far inside the 2e-2 budget (host
    # does the final dots in float64). (walrus rejects matmul PSUM writes
    # at nonzero start partitions, so S stays [16, 1024] in one band.)
    out_S = nc.dram_tensor("out_S", [B_CORE, D], F16, kind="ExternalOutput").ap()

    ctx = contextlib.ExitStack()
    with ctx:
        const = ctx.enter_context(tc.tile_pool(name="const", bufs=1))
        boxpool = ctx.enter_context(tc.tile_pool(name="boxpool", bufs=12))
        sqpool = ctx.enter_context(tc.tile_pool(name="sqpool", bufs=2))
        smpool = ctx.enter_context(tc.tile_pool(name="smpool", bufs=6))
        psS = ctx.enter_context(tc.tile_pool(name="psS", bufs=1, space="PSUM"))
        psmisc = ctx.enter_context(tc.tile_pool(name="psmisc", bufs=1, space="PSUM"))

        # --- DMAs first so the box stream owns the queue from t=0 ---------
        # Box tiles lead (the stream is the critical resource); iou slots in
        # after tile 1 — the mask/coef preamble isn't needed until the first
        # batch's weights anyway. The last two tiles arrive as 4 quarter
        # DMAs each so their sum-of-squares can start before the full tile
        # has landed, shortening the post-stream tail.
        iou_sb = const.tile([128, NT], F32)
        tiles = []
        HW = D // 2
        # per-tile square split between ACT (leading columns) and DVE
        # (trailing columns); asymmetric for the last tile so DVE's faster
        # reduce covers only the very last-arriving bytes
        # 640/384 balances ACT (square+accum is one fused op) against DVE
        # (square is a TT + reduce pair since tensor_tensor_reduce crashes
        # the runtime)
        CUT = [640] * NT
        for t in range(NT):
            bt = boxpool.tile([128, D], F32R, name=f"bt{t}", tag="bt")
            src = box[t * 128:(t + 1) * 128, :].bitcast(F32R)
            if t >= NT - 2:
                # split the trailing tiles' DMAs at the square cut so each
                # engine's piece can start as soon as its columns land
                c = CUT[t]
                nc.sync.dma_start(bt[:, 0:c], src[:, 0:c])
                nc.sync.dma_start(bt[:, c:D], src[:, c:D])
            else:
                nc.sync.dma_start(bt[:], src)
            tiles.append(bt)
            if t == 1:
                nc.sync.dma_start(iou_sb[:], iou_t[:])

        # sparse per-tile weight columns: w_sp[:, 16*t + t//2] nonzero.
        # float32r so the fp32r matmul sees pre-rounded producers (the BIR
        # verifier rejects f32 writers into f32r matmul operands); Memset
        # cannot emit float32r, so zero via a DVE copy-convert from a
        # memset f32 scratch — off the DMA queue, unlike the old DMA fill.
        w_sp = const.tile([128, NT * B_CORE], F32R)
        w_zero = const.tile([128, NT * B_CORE], F32)
        nc.vector.memset(w_zero[:], 0.0)
        nc.vector.tensor_copy(w_sp[:], w_zero[:])

        # --- mask / counts / coefficients ---------------------------------
        # bf16 for the tiny count/broadcast matmuls: walrus codegen rejects
        # the fp32 lowering of K=1/M=1 matmuls, and bf16 is exact for
        # ones/0-1 masks while coef rounding (~4e-3) is far below tolerance.
        ones_col = const.tile([128, 1], BF16)
        nc.vector.memset(ones_col[:], 1.0)
        ones_row = const.tile([1, 128], BF16)
        nc.vector.memset(ones_row[:], 1.0)

        # mask[p, t] = iou >= thres  (1.0 / 0.0)
        mask = const.tile([128, NT], BF16)
        nc.vector.tensor_scalar(mask[:], iou_sb[:], IOU_THRES, None, ALU.is_ge)

        # cnt per row-tile column: ones[128,1].T @ mask -> [1, NT]
        ps_cnt = psmisc.tile([1, NT], F32)
        nc.tensor.matmul(ps_cnt[:], ones_col[:], mask[:], start=True, stop=True)

        cnt_t = const.tile([1, NT], F32)
        nc.vector.tensor_copy(cnt_t[:], ps_cnt[:])
        cnt_pos = const.tile([1, B_CORE], F32)
        nc.vector.tensor_tensor(
            cnt_pos[:], cnt_t[0:1, 0:NT:2], cnt_t[0:1, 1:NT:2], ALU.add
        )
        rcp_p = const.tile([1, B_CORE], F32)
        nc.vector.reciprocal(rcp_p[:], cnt_pos[:])
        cnt_neg = const.tile([1, B_CORE], F32)
        nc.vector.tensor_scalar(
            cnt_neg[:], cnt_pos[:], -1.0, float(P), ALU.mult, ALU.add
        )
        rcp_n = const.tile([1, B_CORE], F32)
        nc.vector.reciprocal(rcp_n[:], cnt_neg[:])

        # coefA=(rcp_p+rcp_n)/T at cols 2b,2b+1 ; coefB=rcp_n/T at NT+...
        coef_row = const.tile([1, 2 * NT], BF16)
        tmp_ab = const.tile([1, B_CORE], F32)
        nc.vector.tensor_tensor(tmp_ab[:], rcp_p[:], rcp_n[:], ALU.add)
        for rep in range(2):
            nc.vector.tensor_scalar(
                coef_row[0:1, rep:NT:2], tmp_ab[:], 1.0 / TEMP, None, ALU.mult
            )
            nc.vector.tensor_scalar(
                coef_row[0:1, NT + rep:2 * NT:2], rcp_n[:], 1.0 / TEMP,
                None, ALU.mult,
            )

        # broadcast to all 128 partitions: ones[1,128].T @ coef[1,2NT]
        ps_coef = psmisc.tile([128, 2 * NT], F32)
        nc.tensor.matmul(
            ps_coef[:], ones_row[:], coef_row[:], start=True, stop=True
        )
        coef_bc = const.tile([128, 2 * NT], F32)
        nc.vector.tensor_copy(coef_bc[:], ps_coef[:])

        # pre_w[p,t] = mask*coefA - coefB: everything but the invnorm factor,
        # computed once so the per-batch tail chain is a single multiply
        pre_w = const.tile([128, NT], F32)
        nc.vector.tensor_tensor(
            pre_w[:], mask[:], coef_bc[:, 0:NT], ALU.mult
        )
        nc.vector.tensor_tensor(
            pre_w[:], pre_w[:], coef_bc[:, NT:2 * NT], ALU.subtract
        )

        # --- main streaming pass over box, one tile at a time -------------
        # Per tile: sum-of-squares split across ACT (first half-D, fused
        # accum) and DVE (second half, fused tensor_tensor_reduce); the two
        # partials combine inside the ACT sqrt via its bias operand, so the
        # whole norm chain is sq(ACT)/sq(DVE) -> sqrt(ACT) -> recip(DVE) ->
        # weight TT(DVE) -> 2 matmuls, with single cross-engine hops.
        ssa = const.tile([128, NT], F32)
        ssb = const.tile([128, NT], F32)
        norm_all = const.tile([128, NT], F32)
        ps_S = psS.tile([B_CORE, D], F32)
        S_sb = const.tile([B_CORE, D], F16)
        for t in range(NT):
            bt = tiles[t]
            c = CUT[t]
            sq = sqpool.tile([128, 640], F32, name="sq", tag="sq")
            nc.scalar.activation(
                sq[:, 0:c], bt[:, 0:c].bitcast(F32), AF.Square,
                accum_out=ssa[:, t:t + 1],
            )
            sq2 = sqpool.tile([128, HW], F32, name="sq2", tag="sq2")
            if USE_TTR:
                nc.vector.tensor_tensor_reduce(
                    sq2[:, 0:D - c], bt[:, c:D].bitcast(F32), bt[:, c:D].bitcast(F32),
                    1.0, 0.0, ALU.mult, ALU.add, accum_out=ssb[:, t:t + 1],
                )
            else:
                nc.vector.tensor_tensor(
                    sq2[:, 0:D - c], bt[:, c:D].bitcast(F32), bt[:, c:D].bitcast(F32),
                    ALU.mult,
                )
                nc.vector.reduce_sum(
                    ssb[:, t:t + 1], sq2[:, 0:D - c], axis=mybir.AxisListType.X
                )
            # norm = sqrt(ssa + ssb): ssb rides in via the bias operand
            if USE_BIAS_SQRT:
                nc.scalar.activation(
                    norm_all[:, t:t + 1], ssa[:, t:t + 1], AF.Sqrt,
                    bias=ssb[:, t:t + 1],
                )
            else:
                nc.vector.tensor_tensor(
                    ssa[:, t:t + 1], ssa[:, t:t + 1], ssb[:, t:t + 1], ALU.add
                )
                nc.scalar.activation(
                    norm_all[:, t:t + 1], ssa[:, t:t + 1], AF.Sqrt
                )
            invn1 = smpool.tile([128, 1], F32, name="invn1", tag="invn")
            nc.vector.reciprocal(invn1[:], norm_all[:, t:t + 1])
            col = t * B_CORE + t // 2
            nc.vector.tensor_tensor(
                w_sp[:, col:col + 1], pre_w[:, t:t + 1], invn1[:], ALU.mult
            )
            lhsT = w_sp[:, t * B_CORE:(t + 1) * B_CORE]
            for h in range(2):
                nc.tensor.matmul(
                    ps_S[:, h * HW:(h + 1) * HW],
                    lhsT,
                    bt[:, h * HW:(h + 1) * HW],
                    start=(t == 0),
                    stop=(t == NT - 1),
                    skip_group_check=True,
                )

        # evict + one fp16 DMA
        nc.vector.tensor_copy(S_sb[:], ps_S[:])
        nc.sync.dma_start(out_S[:], S_sb[:])


_NC_CACHE = None


def _get_nc():
    global _NC_CACHE
    if _NC_CACHE is None:
        nc = bacc.Bacc(
            "TRN2", target_bir_lowering=False, debug=False, num_devices=N_CORES
        )
        with tile.TileContext(nc) as tc:
            _emit(tc)
        nc.compile()
        _NC_CACHE = nc
    return _NC_CACHE


def kernel(box_cls_feat_con, crop_feat_con, batch_size, ious, _trace=False):
    nc = _get_nc()
    box = np.ascontiguousarray(np.asarray(box_cls_feat_con, dtype=np.float32))
    iou = np.asarray(ious, dtype=np.float32)
    maps = []
    for c in range(N_CORES):
        rows = slice(c * ROWS, (c + 1) * ROWS)
        maps.append({
            "box": box[rows],
            "iou_t": np.ascontiguousarray(iou[rows].reshape(NT, 128).T),
        })
    res = run_bass_kernel_spmd(nc, maps, core_ids=list(range(N_CORES)), trace=_trace)
    S = np.concatenate(
        [np.asarray(res.results[c]["out_S"]) for c in range(N_CORES)], axis=0
    ).astype(np.float64)  # [B, D]
    z = np.asarray(crop_feat_con, dtype=np.float64)  # [K, B, D]
    z_n = z / np.clip(np.linalg.norm(z, axis=-1, keepdims=True), EPS, None)
    args = np.einsum("kbd,bd->kb", z_n, S)
    L = np.logaddexp(0.0, args).sum(axis=-1)  # softplus + sum over batches
    out = np.float32(L.min() / float(B))
    if _trace:
        kernel._last_results = res
    return np.asarray(out, dtype=np.float32)
